# revision 14
# baseline (speedup 1.0000x reference)
"""Partition-padding kernel for Trainium2, SPMD across 8 NeuronCores.

Problem: node_features [N=500000, D=128] f32, subgraph_indicator [N] int32
(sorted), batch_size B=256. The reference scatters each subgraph's
contiguous row-block into a zero-padded [B, max_n, D] tensor, then drops
partitions whose padded block sums to exactly zero (empty partitions).

Because the indicator is sorted, the computation is pure data movement:
partition b's rows are node_features[offsets[b] : offsets[b]+counts[b]].

Sharding (one SPMD program, 8 cores): 32 consecutive subgraphs per core,
so the [B, max_n, D] output is batch-sharded with no cross-device
traffic. All data-dependent addressing flows through per-core input
tables, never through per-core code:

  per slot b (32 static iterations):
    - load exactly R = ceil(max_n/128)*128 rows (~1.1 MB, full 128 SBUF
      partitions - partial-partition DMAs lose ~30% bandwidth) starting
      at this slot's row offset, read from a per-core `starts` table into
      an SP register (dynamic-offset HWDGE DMA),
    - one in-place DVE multiply with a per-core 0/1 row mask zeroes rows
      >= counts[b] (fully hidden behind DMA),
    - store exactly max_n rows to the slot at a static offset, split into
      a big rectangular DMA (q=max_n//G full partitions, ACT HWDGE ring)
      plus a tiny r-row tail DMA (SWDGE) so slots stay exactly max_n rows
      and no writes overlap (overlapping DRAM writes serialize Tile's
      scheduling).

The host computes counts/offsets, builds the tables, hands each core a
contiguous view of node_features, concatenates the 8 result slabs, and
applies the reference's keep-filter.

Measured: ~186 us HW exec time (69.3 MB of HBM traffic per core at
~373 GB/s, i.e. at the per-core HBM roofline; DVE and the tail DMAs are
fully overlapped).

Notes on this runtime (axon/PJRT): bass runtime-assert instructions
(enable_asserts=True / values_load bounds checks) wedge the device, so
both stay disabled; bounds are guaranteed by construction on the host.
"""

import numpy as np

import concourse.bacc as bacc
import concourse.mybir as mybir
import concourse.tile as tile
from concourse.bass import ds
from concourse.bass_utils import run_bass_kernel_spmd

F32 = mybir.dt.float32
I32 = mybir.dt.int32

N_CORES = 8

_PROGRAM_CACHE: dict = {}
_KERNEL_STATS: dict = {}


def _build_program(n_cores, P_per_core, max_n, Rx, pmin, D, bufs=12):
    """Build + compile the SPMD program for one geometry.

    G = rows per SBUF partition; a [128, G*D] f32 tile holds R = G*128
    rows. q full partitions plus r tail rows make up one max_n-row slot.
    pmin (multiple of 32) is the first SBUF partition whose rows can
    exceed some slot's count; only [pmin:128] needs masking.
    """
    assert D == 128
    G = (max_n + 127) // 128
    nfree = G * D
    R = G * 128
    q, r = divmod(max_n, G)

    nc = bacc.Bacc(
        "TRN2",
        target_bir_lowering=False,
        debug=False,
        enable_asserts=False,
        num_devices=n_cores,
    )
    x = nc.dram_tensor("x", [Rx * D], F32, kind="ExternalInput").ap()
    starts = nc.dram_tensor(
        "starts", [1, P_per_core], I32, kind="ExternalInput"
    ).ap()
    maskt = nc.dram_tensor(
        "maskt", [128, P_per_core * G], F32, kind="ExternalInput"
    ).ap()
    y = nc.dram_tensor(
        "y", [P_per_core * max_n * D], F32, kind="ExternalOutput"
    ).ap()

    with tile.TileContext(nc) as tc:
        with (
            tc.tile_pool(name="const", bufs=1) as cpool,
            tc.tile_pool(name="work", bufs=bufs) as pool,
        ):
            mask_sb = cpool.tile([128, P_per_core * G], F32)
            nc.sync.dma_start(mask_sb[:], maskt[:])
            starts_sb = cpool.tile([1, P_per_core], I32)
            nc.sync.dma_start(starts_sb[:], starts[:])
            for b in range(P_per_core):
                reg = nc.values_load(
                    starts_sb[0:1, b : b + 1],
                    engines=(mybir.EngineType.SP,),
                    min_val=0,
                    max_val=(Rx - R) * D,
                    skip_runtime_bounds_check=True,
                )
                t = pool.tile([128, nfree], F32, tag="t")
                nc.sync.dma_start(t[:], x[ds(reg, R * D)])
                if pmin < 128:
                    m = mask_sb[pmin:128, b * G : (b + 1) * G]
                    m3 = m.rearrange("p (g o) -> p g o", o=1).to_broadcast(
                        [128 - pmin, G, D]
                    )
                    tm = t[pmin:128, :].rearrange("p (g f) -> p g f", f=D)
                    nc.vector.tensor_mul(tm, tm, m3)
                ybase = b * max_n * D
                nc.scalar.dma_start(y[ybase : ybase + q * nfree], t[:q, :])
                if r:
                    nc.gpsimd.dma_start(
                        y[ybase + q * nfree : ybase + max_n * D],
                        t[q : q + 1, : r * D],
                    )
    nc.compile()
    return nc


def _plan(nf, counts, offsets, max_n, B, n_cores):
    """Host-side planning: per-core input maps + geometry."""
    N, D = nf.shape
    G = (max_n + 127) // 128
    R = G * 128
    q, r = divmod(max_n, G)
    P_per_core = B // n_cores
    assert B % n_cores == 0

    # Each core's x window is one contiguous [Rx, D] view; Rx covers the
    # last slot's R-row load in every core.
    ext = []
    for c in range(n_cores):
        base = offsets[P_per_core * c]
        last = offsets[P_per_core * c + P_per_core - 1]
        ext.append(last - base + R)
    Rx = int(max(ext))
    # Rows below counts.min() are real in every slot; engines address SBUF
    # partition windows starting at multiples of 32.
    pmin = int(min(counts.min() // G, q)) // 32 * 32

    rowidx = np.arange(128)[:, None] * G + np.arange(G)[None, :]  # [128, G]
    in_maps = []
    for c in range(n_cores):
        base = int(offsets[P_per_core * c])
        local_starts = (
            offsets[P_per_core * c : P_per_core * (c + 1)] - base
        ).astype(np.int64)
        assert 0 <= local_starts.min() and local_starts.max() + R <= Rx
        starts_elem = (local_starts * D).astype(np.int32)
        ccounts = counts[P_per_core * c : P_per_core * (c + 1)]
        # maskt[p, b*G+g] = 1.0 iff row p*G+g of slot b is a real row.
        maskt = (rowidx[None, :, :] < ccounts[:, None, None]).astype(np.float32)
        maskt = np.ascontiguousarray(
            np.transpose(maskt, (1, 0, 2)).reshape(128, P_per_core * G)
        )
        if base + Rx <= N:
            xwin = nf[base : base + Rx]
        else:
            xwin = np.zeros((Rx, D), np.float32)
            xwin[: N - base] = nf[base:N]
        in_maps.append(
            {
                "x": np.ascontiguousarray(xwin).reshape(-1),
                "starts": starts_elem.reshape(1, P_per_core),
                "maskt": maskt,
            }
        )
    return in_maps, Rx, pmin, P_per_core


def _ensure_axon_hooks_importable():
    """bass_utils imports antenv.axon_hooks when BASS_TRACE is set; the
    module is absent from this image. Install a no-op fallback (tracing
    then degrades gracefully instead of crashing). A test harness may
    pre-install a real hook module; never overwrite it."""
    import sys as _sys

    if "antenv.axon_hooks" in _sys.modules:
        return
    try:
        import antenv.axon_hooks  # noqa: F401
    except ImportError:
        import types as _types

        mod = _types.ModuleType("antenv.axon_hooks")
        mod.set_axon_ntff_profile_hook = lambda h: None
        mod.get_axon_ntff_profile_hook = lambda: None
        _sys.modules["antenv.axon_hooks"] = mod


def kernel(node_features, subgraph_indicator, batch_size):
    _ensure_axon_hooks_importable()
    nf = np.ascontiguousarray(np.asarray(node_features), dtype=np.float32)
    si = np.asarray(subgraph_indicator).astype(np.int64)
    B = int(batch_size)
    N, D = nf.shape

    counts = np.bincount(si, minlength=B).astype(np.int64)
    offsets = (np.cumsum(counts) - counts).astype(np.int64)
    max_n = int(counts.max())

    in_maps, Rx, pmin, P_per_core = _plan(nf, counts, offsets, max_n, B, N_CORES)

    key = (N_CORES, P_per_core, max_n, Rx, pmin, D)
    nc = _PROGRAM_CACHE.get(key)
    if nc is None:
        nc = _build_program(N_CORES, P_per_core, max_n, Rx, pmin, D)
        _PROGRAM_CACHE[key] = nc

    try:
        res = run_bass_kernel_spmd(nc, in_maps, core_ids=list(range(N_CORES)))
    except Exception:
        # Transient NRT_EXEC_UNIT_UNRECOVERABLE wedges clear on retry.
        import time as _time

        _time.sleep(20)
        res = run_bass_kernel_spmd(nc, in_maps, core_ids=list(range(N_CORES)))
    _KERNEL_STATS["exec_time_ns"] = res.exec_time_ns
    _KERNEL_STATS["mean_exec_time_ns"] = res.mean_exec_time_ns
    _KERNEL_STATS["results"] = res

    padded = np.concatenate(
        [r["y"].reshape(P_per_core, max_n, D) for r in res.results], axis=0
    )  # [B, max_n, D]

    # Reference drops partitions whose padded block sums to exactly zero
    # (only empty partitions, in practice).
    sums = np.zeros(B, np.float32)
    for b in range(B):
        if counts[b]:
            sums[b] = nf[offsets[b] : offsets[b] + counts[b]].sum(dtype=np.float32)
    keep = np.flatnonzero(sums != 0)
    return np.ascontiguousarray(padded[keep])
